# revision 1
# baseline (speedup 1.0000x reference)
import sys

sys.path.insert(0, "/opt/trn_rl_repo")
import numpy as np

import concourse.bass as bass
import concourse.tile as tile
from concourse import bacc, mybir
from concourse.bass_utils import run_bass_kernel_spmd
from concourse.masks import make_identity

f32 = mybir.dt.float32
f32r = mybir.dt.float32r
bf16 = mybir.dt.bfloat16
Exp = mybir.ActivationFunctionType.Exp
AX = mybir.AxisListType.X

B, N, D = 4, 4096, 64
NCORES = 8
NQ = 2048          # queries per core (half a batch)
NK = 4096          # keys per core
QT = NQ // 128     # 16 q-tiles
QB = NQ // 512     # 4 q-blocks
CH = NK // 128     # 32 k-chunks
SCALE = 64.0       # sqrt(N)

_cached = {}


def build_program():
    nc = bacc.Bacc("TRN2", target_bir_lowering=False, debug=False, num_devices=NCORES)
    q_d = nc.dram_tensor("q", [NQ, D], f32, kind="ExternalInput").ap()
    k_d = nc.dram_tensor("k", [NK, D], f32, kind="ExternalInput").ap()
    v_d = nc.dram_tensor("v", [NK, D], f32, kind="ExternalInput").ap()
    o_d = nc.dram_tensor("o", [NQ, D], f32, kind="ExternalOutput").ap()
    o3 = o_d.rearrange("(t p) d -> p t d", p=128)

    with tile.TileContext(nc) as tc:
        import contextlib

        ctx = contextlib.ExitStack()
        with ctx:
            const = ctx.enter_context(tc.tile_pool(name="const", bufs=1))
            big = ctx.enter_context(tc.tile_pool(name="big", bufs=1))
            stage = ctx.enter_context(tc.tile_pool(name="stage", bufs=6))
            attnp = ctx.enter_context(tc.tile_pool(name="attnp", bufs=6))
            pshared = ctx.enter_context(
                tc.tile_pool(name="pshared", bufs=3, space="PSUM")
            )
            pout = ctx.enter_context(tc.tile_pool(name="pout", bufs=2, space="PSUM"))

            ident_f32 = const.tile([128, 128], f32)
            make_identity(nc, ident_f32[:])
            ident_bf = const.tile([128, 128], bf16)
            make_identity(nc, ident_bf[:])
            neg1 = const.tile([1, NK], bf16)
            nc.gpsimd.memset(neg1[:], -1.0)

            k_pack_a = big.tile([128, CH * 128], bf16)
            k_pack_b = big.tile([128, CH * 128], bf16)
            q_pack_h = big.tile([128, NQ], bf16)
            q_pack_l = big.tile([128, NQ], bf16)
            qh_keep = big.tile([128, QT * D], bf16)
            kh2 = big.tile([128, CH * 128], bf16)
            k_nat = big.tile([128, CH * D], f32)
            q_nat = big.tile([128, QT * D], f32)
            v_tmp = big.tile([128, CH * D], f32)
            v_pack = big.tile([128, CH, 65], f32r)
            M_all = big.tile([128, QT], f32)
            Mtmp8 = big.tile([128, 8, QT], f32)
            out_all = big.tile([128, QT * D], f32)

            # ---- bulk input DMAs (natural layouts, one DMA each, SP ring)
            k3o = k_nat[:].rearrange("p (c d) -> p c d", d=D)
            k3i = k_d.rearrange("(c p) d -> p c d", p=128)
            for g in range(4):
                nc.sync.dma_start(out=k3o[:, g * 8 : (g + 1) * 8, :],
                                  in_=k3i[:, g * 8 : (g + 1) * 8, :])
            q3o = q_nat[:].rearrange("p (t d) -> p t d", d=D)
            q3i = q_d.rearrange("(t p) d -> p t d", p=128)
            for g in range(2):
                nc.sync.dma_start(out=q3o[:, g * 8 : (g + 1) * 8, :],
                                  in_=q3i[:, g * 8 : (g + 1) * 8, :])
            nc.sync.dma_start(
                out=v_tmp[:].rearrange("p (c d) -> p c d", d=D),
                in_=v_d.rearrange("(c p) d -> p c d", p=128),
            )

            # ---- k prep: staging [kh | kl] -> PE transpose (batched 4/bank)
            for g in range(CH // 4):
                p_t = pshared.tile([128, 512], bf16, tag="ps")
                for i in range(4):
                    c = g * 4 + i
                    kst = stage.tile([128, 128], bf16, tag="st")
                    src = k_nat[:, c * D : (c + 1) * D]
                    nc.vector.tensor_copy(kst[:, 0:64], src)
                    nc.vector.tensor_sub(kst[:, 64:128], src, kst[:, 0:64])
                    nc.tensor.transpose(
                        p_t[:, i * 128 : (i + 1) * 128], kst[:], ident_bf[:]
                    )
                nc.scalar.copy(k_pack_a[:, g * 512 : (g + 1) * 512], p_t[:])
            nc.vector.tensor_copy(k_pack_b[0:127, :], k_pack_a[0:127, :])
            nc.sync.dma_start(out=k_pack_b[127:128, :], in_=neg1[0:1, :])
            nc.vector.tensor_copy(kh2[0:64, :], k_pack_a[0:64, :])
            nc.vector.tensor_copy(kh2[64:128, :], k_pack_a[0:64, :])

            # ---- q hi prep: staging [qh | qh] -> PE transpose (batched 4/bank)
            for g in range(QT // 4):
                p_t = pshared.tile([128, 512], bf16, tag="ps")
                for i in range(4):
                    t = g * 4 + i
                    qsh = stage.tile([128, 128], bf16, tag="st")
                    src = q_nat[:, t * D : (t + 1) * D]
                    nc.vector.tensor_copy(qh_keep[:, t * D : (t + 1) * D], src)
                    nc.gpsimd.tensor_copy(qsh[:, 0:64], qh_keep[:, t * D : (t + 1) * D])
                    nc.gpsimd.tensor_copy(qsh[:, 64:128], qsh[:, 0:64])
                    nc.tensor.transpose(
                        p_t[:, i * 128 : (i + 1) * 128], qsh[:], ident_bf[:]
                    )
                nc.scalar.copy(q_pack_h[:, g * 512 : (g + 1) * 512], p_t[:])

            # ---- v' pack: [v | 1] per chunk, cast to f32r (needed from block0)
            v3 = v_tmp[:].rearrange("p (c d) -> p c d", c=CH)
            nc.vector.tensor_copy(v_pack[:, :, 0:64], v3)
            nc.vector.memset(v_pack[:, :, 64:65].bitcast(f32), 1.0)

            def lo_prep(g):
                p_t = pshared.tile([128, 512], bf16, tag="ps")
                for i in range(4):
                    t = g * 4 + i
                    qsl = stage.tile([128, 128], bf16, tag="st")
                    src = q_nat[:, t * D : (t + 1) * D]
                    nc.vector.tensor_sub(
                        qsl[:, 0:64], src, qh_keep[:, t * D : (t + 1) * D]
                    )
                    nc.gpsimd.tensor_copy(qsl[:, 64:127], qsl[:, 0:63])
                    nc.vector.tensor_copy(qsl[:, 127:128], M_all[:, t : t + 1])
                    nc.tensor.transpose(
                        p_t[:, i * 128 : (i + 1) * 128], qsl[:], ident_bf[:]
                    )
                nc.vector.tensor_copy(q_pack_l[:, g * 512 : (g + 1) * 512], p_t[:])

            def block(blk):
                qs = slice(blk * 512, (blk + 1) * 512)
                p_o = pout.tile([65, 512], f32, tag="po")
                for cc in range(CH // 2):
                    p_s = pshared.tile([128, 1024], f32, tag="ps")
                    for h in range(2):
                        c = cc * 2 + h
                        ks = slice(c * 128, (c + 1) * 128)
                        nc.tensor.matmul(
                            p_s[:, h * 512 : (h + 1) * 512],
                            k_pack_a[:, ks],
                            q_pack_h[:, qs],
                            start=True,
                            stop=False,
                        )
                        nc.tensor.matmul(
                            p_s[:, h * 512 : (h + 1) * 512],
                            k_pack_b[:, ks],
                            q_pack_l[:, qs],
                            start=False,
                            stop=True,
                        )
                    at = attnp.tile([128, 1024], f32r, tag="at")
                    nc.scalar.activation(
                        out=at[:], in_=p_s[:], func=Exp, bias=0.0, scale=SCALE
                    )
                    for h in range(2):
                        c = cc * 2 + h
                        nc.tensor.matmul(
                            p_o[:],
                            v_pack[:, c, :],
                            at[:, h * 512 : (h + 1) * 512],
                            start=(c == 0),
                            stop=(c == CH - 1),
                        )
                # epilogue: evict, transpose, normalize, store
                O_sb = stage.tile([65, 512], f32, tag="ob")
                nc.vector.tensor_copy(O_sb[:], p_o[:])
                for j in range(4):
                    t_idx = blk * 4 + j
                    p_T = pout.tile([128, 65], f32, tag="po")
                    nc.tensor.matmul(
                        p_T[:],
                        O_sb[:, j * 128 : (j + 1) * 128],
                        ident_f32[0:65, 0:65],
                        is_transpose=True,
                    )
                    rZ = stage.tile([128, 1], f32, tag="rz")
                    nc.vector.reciprocal(rZ[:], p_T[:, 64:65])
                    nc.vector.tensor_scalar_mul(
                        out_all[:, t_idx * D : (t_idx + 1) * D], p_T[:, 0:64], rZ[:]
                    )
                nc.sync.dma_start(
                    out=o3[:, blk * 4 : (blk + 1) * 4, :],
                    in_=out_all[:, blk * 4 * D : (blk + 1) * 4 * D].rearrange(
                        "p (t d) -> p t d", d=D
                    ),
                )

            def mp_round(t, r):
                # two concurrent K=64 matmuls in PE row-groups 0/1:
                # tile t from partitions 0:64, tile t+1 from partitions 64:128
                col = r * 512
                p_m = pshared.tile([128, 1024], f32, tag="ps")
                nc.tensor.matmul(
                    p_m[:, 0:512],
                    q_pack_h[0:64, t * 128 : (t + 1) * 128],
                    kh2[0:64, col : col + 512],
                    start=True,
                    stop=True,
                )
                nc.tensor.matmul(
                    p_m[:, 512:1024],
                    q_pack_h[64:128, (t + 1) * 128 : (t + 2) * 128],
                    kh2[64:128, col : col + 512],
                    start=True,
                    stop=True,
                )
                nc.vector.reduce_max(
                    Mtmp8[:, r, t : t + 2],
                    p_m[:].rearrange("p (g x) -> p g x", g=2),
                    axis=AX,
                )

            def maxpass_gen(g):
                for tp in range(2):
                    t = 4 * g + tp * 2
                    for r in range(8):
                        mp_round(t, r)
                        yield
                for t in range(4 * g, 4 * g + 4):
                    nc.vector.reduce_max(M_all[:, t : t + 1], Mtmp8[:, :, t], axis=AX)

            def block_gen(blk):
                qs = slice(blk * 512, (blk + 1) * 512)
                p_o = pout.tile([65, 512], f32, tag="po")
                for cc in range(CH // 2):
                    p_s = pshared.tile([128, 1024], f32, tag="ps")
                    for h in range(2):
                        c = cc * 2 + h
                        ks = slice(c * 128, (c + 1) * 128)
                        nc.tensor.matmul(
                            p_s[:, h * 512 : (h + 1) * 512],
                            k_pack_a[:, ks],
                            q_pack_h[:, qs],
                            start=True,
                            stop=False,
                        )
                        nc.tensor.matmul(
                            p_s[:, h * 512 : (h + 1) * 512],
                            k_pack_b[:, ks],
                            q_pack_l[:, qs],
                            start=False,
                            stop=True,
                        )
                    at = attnp.tile([128, 1024], f32r, tag="at")
                    nc.scalar.activation(
                        out=at[:], in_=p_s[:], func=Exp, bias=0.0, scale=SCALE
                    )
                    for h in range(2):
                        c = cc * 2 + h
                        nc.tensor.matmul(
                            p_o[:],
                            v_pack[:, c, :],
                            at[:, h * 512 : (h + 1) * 512],
                            start=(c == 0),
                            stop=(c == CH - 1),
                        )
                    yield
                O_sb = stage.tile([65, 512], f32, tag="ob")
                nc.vector.tensor_copy(O_sb[:], p_o[:])
                for j in range(4):
                    t_idx = blk * 4 + j
                    p_T = pout.tile([128, 65], f32, tag="po")
                    nc.tensor.matmul(
                        p_T[:],
                        O_sb[:, j * 128 : (j + 1) * 128],
                        ident_f32[0:65, 0:65],
                        is_transpose=True,
                    )
                    rZ = stage.tile([128, 1], f32, tag="rz")
                    nc.vector.reciprocal(rZ[:], p_T[:, 64:65])
                    nc.vector.tensor_scalar_mul(
                        out_all[:, t_idx * D : (t_idx + 1) * D], p_T[:, 0:64], rZ[:]
                    )
                nc.sync.dma_start(
                    out=o3[:, blk * 4 : (blk + 1) * 4, :],
                    in_=out_all[:, blk * 4 * D : (blk + 1) * 4 * D].rearrange(
                        "p (t d) -> p t d", d=D
                    ),
                )
                yield

            # driver: phase0 = round-major mp tiles 0-3 (starts as soon as
            # the first k chunks are packed); then interleave mp(g)/block(g-1)
            for r in range(8):
                for tp in range(2):
                    t = tp * 2
                    mp_round(t, r)
            for t in range(4):
                nc.vector.reduce_max(M_all[:, t : t + 1], Mtmp8[:, :, t], axis=AX)
            lo_prep(0)
            def run_interleaved(gens):
                alive = list(gens)
                while alive:
                    for gen, ratio in list(alive):
                        for _ in range(ratio):
                            try:
                                next(gen)
                            except StopIteration:
                                alive.remove((gen, ratio))
                                break

            run_interleaved([(maxpass_gen(1), 1), (block_gen(0), 1)])
            lo_prep(1)

            def mp23():
                yield from maxpass_gen(2)
                lo_prep(2)
                yield from maxpass_gen(3)

            run_interleaved([(mp23(), 2), (block_gen(1), 1)])
            lo_prep(3)
            run_interleaved([(block_gen(2), 1), (block_gen(3), 1)])

    nc.compile()
    return nc


def kernel(q, k, v):
    if "nc" not in _cached:
        _cached["nc"] = build_program()
    nc = _cached["nc"]
    in_maps = []
    for c in range(NCORES):
        b, h = c // 2, c % 2
        in_maps.append(
            {
                "q": np.ascontiguousarray(q[b, h * NQ : (h + 1) * NQ, :]),
                "k": np.ascontiguousarray(k[b]),
                "v": np.ascontiguousarray(v[b]),
            }
        )
    res = run_bass_kernel_spmd(nc, in_maps, list(range(NCORES)))
    out = np.empty((B, N, D), dtype=np.float32)
    for c in range(NCORES):
        b, h = c // 2, c % 2
        out[b, h * NQ : (h + 1) * NQ, :] = res.results[c]["o"]
    return out

